# revision 74
# baseline (speedup 1.0000x reference)
"""Trainium2 Bass kernel for nn_AttentionModel_63737314672806.

Sharding: data-parallel over batch (B=128) across 8 NeuronCores; each core
processes 16 batch elements (2048 tokens) through the full model. Weights are
replicated (broadcast) to every core. No collectives.

Device layout: activations are kept feature-major ("transposed"):
  xT[p, c, t] = x[token t, feature c*128+p]   (SBUF tile [128, 8, 2048])
so every dense layer is psum[dout, tok] = sum_kc matmul(lhsT=W[kc, dout_chunk],
rhs=xT[kc, tok_tile]) and the output is feature-major again (no transposes).

Precision: the large projections/FFN matmuls run in fp8 e4m3 with
perf_mode=DoubleRow (2 fp8 weights per PE cell -> 2x throughput). Weights are
host-quantized with power-of-2 per-matrix scales; descales fold into existing
epilogue constants. Residual/LayerNorm paths are bf16; statistics f32.

Pipeline structure (v2): each layer's attention half is emitted as a 4-deep
software pipeline over the 4 token groups --
  stage s:  stats(s-3-as-needed) | qkv-proj(s) | attention(s-1) |
            o/g-proj+residual(s-2) | LN-rows/apply/copies(s-3)
so the in-order PE queue never holds an instruction whose dependencies are
less than a full stage old.  The relative-position bias is accumulated into
the QK^T PSUM with one identity-RHS matmul per batch (replacing the
exp-table multiply); LayerNorm statistics accumulate into partition-stacked
PSUM rows via ones-vector matmuls emitted a full stage after their inputs;
LN row math runs on [2,512] tiles (Act Rsqrt for 1/sqrt(var+eps)); the LN
apply is 3 cheap bf16 SBUF ops per chunk against Pool-broadcast rs/m*rs
rows; squares for sum(x^2) run on the otherwise-idle Pool engine.
"""

import math

import numpy as np
import ml_dtypes

import concourse.bass as bass
import concourse.bacc as bacc
import concourse.mybir as mybir
import concourse.tile as tile
from concourse.bass_utils import run_bass_kernel_spmd

BF16 = mybir.dt.bfloat16
F8 = mybir.dt.float8e4
F32 = mybir.dt.float32
F32R = mybir.dt.float32r
AF = mybir.ActivationFunctionType
OP = mybir.AluOpType
DR = mybir.MatmulPerfMode.DoubleRow

NCORES = 8
B = 128
L = 128
DFEAT = 32
H = 8
DK = 128
D = 1024  # = H * DK
FF = 4096
NL = 2
MAXPOS = 128
OTHER = 64
EPS = 1e-6

BPC = B // NCORES       # 16 batches per core
NTOK = BPC * L          # 2048 tokens per core
NG = 4                  # batch groups per core (4 batches / 512 tokens each)
GB = BPC // NG          # batches per group = 4
GT = GB * L             # tokens per group = 512
DC = D // 128           # 8 feature chunks
FC = FF // 128          # 32 ff chunks
QSCALE = 1.0 / math.sqrt(float(DK))
SURVIVE = [1.0, 0.5]    # jnp.linspace(1.0, 0.5, 2)

# fp8 scale plan (all powers of 2; folded into epilogue constants)
SW = 1024.0             # weight scale: wq wk wv wg wo wfg wf2
SW1 = 32.0              # weight scale for wf1 (its descale rides on f8)
SQ = 32.0               # q stored as 32*q
SV = 32.0               # v stored as 32*v
SAO = 32.0              # ao stored as 32*ao  (SV==SAO makes rb == 1/sm)
SF = SW1                # f stored as 32*f

CQ = SQ * QSCALE / SW
CK = 1.0 / SW
CV = SV / SW
CO = 1.0 / (SW * SAO)
CG = 1.0 / SW
CFG = 1.0 / SW
CF2 = 1.0 / (SW * SF)
EXPS = 1.0 / SQ

_cached = {}


def _build_nc():
    nc = bacc.Bacc("TRN2", target_bir_lowering=False, debug=False,
                   num_devices=NCORES)

    def din(name, shape, dtype):
        return nc.dram_tensor(name, list(shape), dtype, kind="ExternalInput")

    t = {}
    t["cgmT"] = din("cgmT", [DFEAT, NTOK], BF16)
    t["w_in"] = din("w_in", [DFEAT, D], BF16)
    t["b_in_c"] = din("b_in_c", [128, DC], F32)
    for w in ("wq", "wk", "wv"):
        t[w] = din(w, [NL, 128, DC, D], F8)
    # wo+wg fused, chunked on the output dim so each DMA is contiguous
    t["wog"] = din("wog", [NL, DC, 128, 2, DC, 128], F8)
    # wf1+wfg fused, 2 ff-chunks (256 cols) per load
    t["wf12"] = din("wf12", [NL, FC // 2, 128, 2, DC, 256], F8)
    # wf2 per-output-chunk contiguous loads
    t["wf2"] = din("wf2", [NL, DC, 128, FC, 128], F8)
    for bn in ("bq_c", "bk_c", "bg_c", "bo_c", "bf2_c",
               "ln1_s_c", "ln1_b_c", "ln2_s_c", "ln2_b_c"):
        t[bn] = din(bn, [128, NL, DC], F32)
    t["bf1_c"] = din("bf1_c", [128, NL, FC], F32)
    t["bfg_c"] = din("bfg_c", [128, NL, FC], F32)
    # per-batch rel-pos bias, pre-transposed for the identity-matmul add:
    # battn[q, i, b, k] = SQ * rel_emb[i][b_global - q + 127, k]
    t["battn"] = din("battn", [128, NL, BPC, 128], BF16)
    t["ident4"] = din("ident4", [128, 512], BF16)
    t["wd1"] = din("wd1", [128, 17, 128], BF16)
    t["bd1_c"] = din("bd1_c", [128, 1], F32)
    t["ln3_s_c"] = din("ln3_s_c", [128, 1], F32)
    t["ln3_b_c"] = din("ln3_b_c", [128, 1], F32)
    t["ln3_s_row"] = din("ln3_s_row", [1, 128], F32R)
    t["wd2"] = din("wd2", [128, 128], BF16)
    t["bd2_c"] = din("bd2_c", [128, 1], F32)
    t["wout"] = din("wout", [128, 1], BF16)
    t["bout_t"] = din("bout_t", [1, 1], F32)
    t["otherT"] = din("otherT", [128, BPC], BF16)
    t["onesc"] = din("onesc", [128, 1], F32R)
    t["onesr"] = din("onesr", [1, 128], F32R)
    y_out = nc.dram_tensor("y", [1, BPC], F32, kind="ExternalOutput")

    with tile.TileContext(nc, pool_alloc_mode="queue") as tc:
        _emit(nc, tc, t, y_out)
    nc.compile()
    return nc


def _emit(nc, tc, t, y_out):
    with (
        tc.tile_pool(name="persist", bufs=1) as pp,
        tc.tile_pool(name="dramp", bufs=1, space="DRAM") as dp,
        tc.tile_pool(name="mm_psum", bufs=6, space="PSUM") as mmp,
        tc.tile_pool(name="sum_psum", bufs=1, space="PSUM") as srp,
        tc.tile_pool(name="stat_psum", bufs=1, space="PSUM") as stp,
    ):
        # ---- persistent SBUF state (per-group quarters so the Tile
        # dependency tracker never serializes on unrelated group writes) ----
        xT = [pp.tile([128, DC, GT], BF16, name=f"xT{g}") for g in range(NG)]
        xT8 = [pp.tile([128, DC, GT], F8, name=f"xT8{g}") for g in range(NG)]
        h8 = [pp.tile([128, DC, GT], F8, name=f"h8{g}") for g in range(NG)]
        ones_col_bf = pp.tile([128, 1], BF16)
        nc.vector.memset(ones_col_bf, 1.0)
        ones_col_8 = pp.tile([128, 1], F8)
        nc.vector.memset(ones_col_8, 1.0)
        ones_col2_8 = pp.tile([128, 2, 1], F8)
        nc.vector.memset(ones_col2_8, 1.0)
        ones_row_bf = pp.tile([1, 128], BF16)
        nc.vector.memset(ones_row_bf, 1.0)
        ones_col_f = pp.tile([128, 1], F32R)
        nc.sync.dma_start(out=ones_col_f[:], in_=t["onesc"][:])
        ones_row_f = pp.tile([1, 128], F32R)
        nc.sync.dma_start(out=ones_row_f[:], in_=t["onesr"][:])
        eps1 = pp.tile([1, 1], F32)
        nc.vector.memset(eps1, EPS)
        pool_sum = pp.tile([128, DC, BPC], BF16)
        pool_max = pp.tile([128, DC, BPC], BF16)

        # ---- input projection: xT = cgm @ W_in + b_in ----
        with tc.tile_pool(name="inproj", bufs=1) as ip:
            cgmT_s = ip.tile([DFEAT, NTOK], BF16, name="e_cgmT")
            nc.gpsimd.dma_start(out=cgmT_s[:], in_=t["cgmT"][:])
            w_in_s = ip.tile([DFEAT, D], BF16, name="e_w_in")
            nc.gpsimd.dma_start(out=w_in_s[:], in_=t["w_in"][:])
            ident4_s = pp.tile([128, 512], BF16, name="c_ident4")
            nc.gpsimd.dma_start(out=ident4_s[:], in_=t["ident4"][:])
            battn_s = pp.tile([128, NL, BPC, 128], BF16, name="c_battn")
            nc.gpsimd.dma_start(out=battn_s[:], in_=t["battn"][:])

            # small constants from DRAM
            consts = {}
            for name in ("b_in_c", "bq_c", "bk_c", "bg_c", "bo_c", "bf2_c",
                         "ln1_s_c", "ln1_b_c", "ln2_s_c", "ln2_b_c",
                         "bf1_c", "bfg_c",
                         "bd1_c", "ln3_s_c", "ln3_b_c",
                         "ln3_s_row", "wd2", "bd2_c", "wout", "bout_t",
                         "otherT"):
                ap = t[name]
                tl = pp.tile(list(ap.shape), ap.dtype, name=f"c_{name}")
                nc.sync.dma_start(out=tl[:], in_=ap[:])
                consts[name] = tl
            consts["ident4"] = ident4_s

            _inproj(nc, tc, t, consts, cgmT_s, w_in_s, xT, xT8, mmp)

        h_dram = [dp.tile([128, DC, GT], BF16, name=f"hd{g}")
                  for g in range(NG)]

        shared = dict(
            xT=xT, xT8=xT8, h8=h8, h_dram=h_dram,
            ones_col_bf=ones_col_bf, ones_col_8=ones_col_8,
            ones_row_bf=ones_row_bf, ones_col_f=ones_col_f,
            ones_row_f=ones_row_f, eps1=eps1, ones_col2_8=ones_col2_8,
            pool_sum=pool_sum, pool_max=pool_max, battn_s=battn_s,
        )

        # ---- transformer layers ----
        for i in range(NL):
            _layer(nc, tc, t, consts, i, shared, mmp, srp, stp)

        # ---- head ----
        _head(nc, tc, t, consts, shared, mmp, srp, stp, y_out)


def _inproj(nc, tc, t, consts, cgmT_s, w_in_s, xT, xT8, mmp):
    for dd in range(DC):
        for g in range(NG):
            ps = mmp.tile([128, GT], F32, tag="mm")
            nc.tensor.matmul(ps[:], w_in_s[:, dd * 128:(dd + 1) * 128],
                             cgmT_s[:, g * GT:(g + 1) * GT],
                             start=True, stop=True)
            nc.vector.tensor_scalar_add(
                out=xT[g][:, dd, :], in0=ps[:],
                scalar1=consts["b_in_c"][:, dd:dd + 1])
            nc.scalar.copy(
                out=xT8[g][:, dd, :],
                in_=xT[g][:, dd, :])


def _ln_rows(nc, lp, st, inv_d, eps1, tag):
    """From partition-stacked stats st[0]=sum(x), st[32]=sum(x^2) (each
    [1, n]), produce rs16 = 1/sqrt(var+eps) and mrs16 = mean*rs as [1, n]
    bf16 rows (for Pool partition-broadcast)."""
    n = st.shape[-1]
    with nc.allow_low_precision(reason="bf16 LN stats within tolerance"):
        m = lp.tile([1, 512], BF16, tag=f"{tag}m", name="m")[:, :n]
        nc.vector.tensor_scalar_mul(out=m, in0=st[0:1, :], scalar1=inv_d)
        # e = E[x^2] + eps
        e = lp.tile([1, 512], BF16, tag=f"{tag}e", name="e")[:, :n]
        nc.vector.tensor_scalar(out=e, in0=st[32:33, :], scalar1=inv_d,
                                scalar2=EPS, op0=OP.mult, op1=OP.add)
        m2 = lp.tile([1, 512], BF16, tag=f"{tag}t", name="m2")[:, :n]
        nc.vector.tensor_tensor(out=m2, in0=m, in1=m, op=OP.mult)
        var = lp.tile([1, 512], BF16, tag=f"{tag}va", name="var")[:, :n]
        nc.vector.tensor_tensor(out=var, in0=e, in1=m2, op=OP.subtract)
        sd = lp.tile([1, 512], BF16, tag=f"{tag}t", name="sd")[:, :n]
        nc.scalar.activation(out=sd, in_=var, func=AF.Sqrt, scale=1.0)
        rs16 = lp.tile([1, 512], BF16, tag=f"{tag}rs", name="rs16")[:, :n]
        nc.vector.reciprocal(out=rs16, in_=sd)
        mrs16 = lp.tile([1, 512], BF16, tag=f"{tag}mr", name="mrs16")[:, :n]
        nc.vector.tensor_tensor(out=mrs16, in0=m, in1=rs16,
                                op=OP.mult)
    return rs16, mrs16


def _layer(nc, tc, t, consts, i, sh, mmp, srp, stp):
    """One transformer layer. Attention is a 4-deep software pipeline over
    the 4 token groups; the FFN's first half (E = f1/fg+gate for tokens of
    groups 0/1) is emitted BETWEEN the attention pipeline and the LN1
    rows/apply stages of groups 2/3, so the PE streams FFN matmuls while
    DVE finishes the attention tail.  Right-side SBUF pools let the
    attention-early pools close before the FFN pools open."""
    xT, xT8, h8, h_dram = sh["xT"], sh["xT8"], sh["h8"], sh["h_dram"]
    with (
        tc.tile_pool(name="lnrow", bufs=1, side="right") as lrp,
        tc.tile_pool(name="lnsm", bufs=2, side="right") as lp,
    ):
        # res/h-staging pools die right after stageD(3); allocate them
        # innermost on the right stack and release them before the
        # F-phase pools open
        rp = tc.alloc_tile_pool(name="res", bufs=2, side="right")
        hstp = tc.alloc_tile_pool(name="hst", bufs=1, side="right")
        state = {}

        # ------------------------------------------------------------------
        # attention stage emitters
        # ------------------------------------------------------------------
        def stageA(g, wp, gp):
            wq_s, wk_s, wv_s = state["wqkv"]
            x8g = xT8[g]
            qT_g = gp.tile([128, DC, GT], F8, tag="q")
            kT_g = gp.tile([128, DC, GT], F8, tag="k")
            for dd in range(DC):
                psq = mmp.tile([128, GT], F32, tag="mm")
                psk = mmp.tile([128, GT], F32, tag="mm")
                for j in range(DC // 2):
                    nc.tensor.matmul(
                        psq[:], wq_s[:, 2 * j:2 * j + 2,
                                     dd * 128:(dd + 1) * 128],
                        x8g[:, 2 * j:2 * j + 2, :],
                        start=(j == 0), stop=(j == DC // 2 - 1), perf_mode=DR)
                for j in range(DC // 2):
                    nc.tensor.matmul(
                        psk[:], wk_s[:, 2 * j:2 * j + 2,
                                     dd * 128:(dd + 1) * 128],
                        x8g[:, 2 * j:2 * j + 2, :],
                        start=(j == 0), stop=(j == DC // 2 - 1), perf_mode=DR)
                nc.scalar.activation(
                    out=qT_g[:, dd, :], in_=psq[:], func=AF.Identity,
                    bias=consts["bq_c"][:, i, dd:dd + 1], scale=CQ)
                if dd % 2 == 0:
                    nc.scalar.activation(
                        out=kT_g[:, dd, :], in_=psk[:], func=AF.Identity,
                        bias=consts["bk_c"][:, i, dd:dd + 1], scale=CK)
                else:
                    nc.vector.tensor_scalar(
                        out=kT_g[:, dd, :], in0=psk[:], scalar1=CK,
                        scalar2=consts["bk_c"][:, i, dd:dd + 1],
                        op0=OP.mult, op1=OP.add)

            v_g = gp.tile([128, GB, D], F8, tag="v")
            for jj in range(GB):
                btok = slice(jj * L, (jj + 1) * L)
                psv0 = mmp.tile([128, 512], F32, tag="mm", name="psv0")
                psv1 = mmp.tile([128, 512], F32, tag="mm", name="psv1")
                for j in range(DC // 2):
                    for cc, psv in ((0, psv0), (1, psv1)):
                        nc.tensor.matmul(
                            psv[:], x8g[:, 2 * j:2 * j + 2, btok],
                            wv_s[:, 2 * j:2 * j + 2,
                                 cc * 512:(cc + 1) * 512],
                            start=(j == 0), stop=(j == DC // 2 - 1),
                            perf_mode=DR)
                for cc, psv in ((0, psv0), (1, psv1)):
                    nc.vector.tensor_scalar_mul(
                        out=v_g[:, jj, cc * 512:(cc + 1) * 512], in0=psv[:],
                        scalar1=CV)
            state[("qkv", g)] = (qT_g, kT_g, v_g)

        def stageB(g, aop, at):
            qT_g, kT_g, v_g = state.pop(("qkv", g))
            battn_s = sh["battn_s"]
            ao_g = aop.tile([128, DC, GT], F8, tag="ao")
            sums = srp.tile([65, 512], F32, tag="sum", name="sums")
            items = [(jj, hb) for jj in range(GB) for hb in range(2)]
            SKEW = 2
            pend = {}

            def head_part(idx):
                jj, hb = items[idx]
                b_local = g * GB + jj
                jtok = slice(jj * L, (jj + 1) * L)
                pa = mmp.tile([128, 512], F32, tag="mm", name="pa")
                for hx in range(4):
                    hh = hb * 4 + hx
                    nc.tensor.matmul(
                        pa[:, hx * 128:(hx + 1) * 128],
                        kT_g[:, hh, jtok], qT_g[:, hh, jtok],
                        start=(hx == 0), stop=False,
                        skip_group_check=True)
                nc.tensor.matmul(
                    pa[:], battn_s[:, i, b_local, :], consts["ident4"][:],
                    start=False, stop=True, skip_group_check=True)
                awe2 = at.tile([128, 512], F8, tag="awe2")
                nc.scalar.activation(out=awe2[:], in_=pa[:], func=AF.Exp,
                                     scale=EXPS)
                pend[idx] = awe2

            def tail_part(idx):
                jj, hb = items[idx]
                awe2 = pend.pop(idx)
                po = 32 * (idx % 3)
                srow = sums[po:po + 1, :]
                nc.tensor.matmul(srow, sh["ones_col_8"][:, 0:1],
                                 awe2[:], start=True, stop=True,
                                 skip_group_check=True)
                rc = at.tile([1, 512], BF16, tag="rc")
                with nc.allow_low_precision(reason="softmax denom in bf16"):
                    nc.vector.reciprocal(out=rc[:], in_=srow)
                rb = at.tile([128, 512], BF16, tag="rb")
                nc.gpsimd.partition_broadcast(rb[:], rc[:])
                pa2 = mmp.tile([128, 512], F32, tag="mm", name="pa2")
                for hx in range(4):
                    hh = hb * 4 + hx
                    nc.tensor.matmul(
                        pa2[:, hx * 128:(hx + 1) * 128],
                        v_g[:, jj, hh * 128:(hh + 1) * 128],
                        awe2[:, hx * 128:(hx + 1) * 128],
                        start=(hx == 0), stop=(hx == 3),
                        skip_group_check=True)
                jtok = slice(jj * L, (jj + 1) * L)
                nc.vector.tensor_tensor(
                    out=ao_g[:, hb * 4:(hb + 1) * 4, jtok],
                    in0=pa2.rearrange("p (h c) -> p h c", c=128),
                    in1=rb.rearrange("p (h c) -> p h c", c=128),
                    op=OP.mult)

            for idx in range(len(items) + SKEW):
                if idx < len(items):
                    head_part(idx)
                if idx >= SKEW:
                    tail_part(idx - SKEW)
            state[("ao", g)] = ao_g

        def stageC(g, wc, sqp):
            ao_g = state.pop(("ao", g))
            xg, x8g = xT[g], xT8[g]
            res_t = rp.tile([128, DC, GT], BF16, tag="res")
            sq_t = sqp.tile([128, DC, GT], F8, tag="sq")
            for dd in range(DC):
                wog_ch = wc.tile([128, 2, DC, 128], F8, tag="wog")
                nc.sync.dma_start(out=wog_ch[:], in_=t["wog"][i, dd])
                pso = mmp.tile([128, GT], F32, tag="mm")
                psg = mmp.tile([128, GT], F32, tag="mm")
                for j in range(DC // 2):
                    nc.tensor.matmul(pso[:],
                                     wog_ch[:, 0, 2 * j:2 * j + 2, :],
                                     ao_g[:, 2 * j:2 * j + 2, :],
                                     start=(j == 0), stop=(j == DC // 2 - 1),
                                     perf_mode=DR)
                for j in range(DC // 2):
                    nc.tensor.matmul(psg[:],
                                     wog_ch[:, 1, 2 * j:2 * j + 2, :],
                                     x8g[:, 2 * j:2 * j + 2, :],
                                     start=(j == 0), stop=(j == DC // 2 - 1),
                                     perf_mode=DR)
                sig = sqp.tile([128, GT], BF16, tag="sig")
                nc.scalar.activation(out=sig[:], in_=psg[:], func=AF.Sigmoid,
                                     bias=consts["bg_c"][:, i, dd:dd + 1],
                                     scale=CG)
                ot = sqp.tile([128, GT], BF16, tag="ot")
                nc.scalar.activation(out=ot[:], in_=pso[:],
                                     func=AF.Identity,
                                     bias=consts["bo_c"][:, i, dd:dd + 1],
                                     scale=CO)
                nc.vector.tensor_mul(out=res_t[:, dd, :], in0=ot[:],
                                     in1=sig[:])
                nc.vector.tensor_add(out=res_t[:, dd, :],
                                     in0=res_t[:, dd, :], in1=xg[:, dd, :])
                nc.gpsimd.tensor_tensor(out=sq_t[:, dd, :],
                                        in0=res_t[:, dd, :],
                                        in1=res_t[:, dd, :], op=OP.mult)
            state[("res", g)] = (res_t, sq_t)

        def stats(g):
            res_t, sq_t = state[("res", g)]
            st = stp.tile([33, 512], F32, tag="st", name="st1")
            for dd in range(DC):
                nc.tensor.matmul(st[0:1, :],
                                 sh["ones_col_bf"][:, 0:1],
                                 res_t[:, dd, :],
                                 start=(dd == 0), stop=(dd == DC - 1),
                                 skip_group_check=True)
            for dd in range(DC):
                nc.tensor.matmul(st[32:33, :],
                                 sh["ones_col_8"][:, 0:1],
                                 sq_t[:, dd, :],
                                 start=(dd == 0), stop=(dd == DC - 1),
                                 skip_group_check=True)
            state[("st", g)] = st

        def rowsD(g):
            """LN1 rows: consumes the stat PSUM early so the stat ring can
            stay 1-deep; broadcasts land in SBUF for the (later) apply."""
            st = state.pop(("st", g))
            rs16, mrs16 = _ln_rows(nc, lrp, st, 1.0 / D, sh["eps1"], "a")
            rsb = lp.tile([128, GT], BF16, tag="rsb")
            nc.gpsimd.partition_broadcast(rsb[:], rs16)
            mrsb = lp.tile([128, GT], BF16, tag="mrsb")
            nc.gpsimd.partition_broadcast(mrsb[:], mrs16)
            state[("row", g)] = (rsb, mrsb)

        def stageD(g):
            res_t, sq_t = state.pop(("res", g))
            rsb, mrsb = state.pop(("row", g))
            h_g = hstp.tile([128, DC, GT], BF16, tag="hg")
            for dd in range(DC):
                t1 = lp.tile([128, GT], BF16, tag="t1")
                nc.vector.tensor_tensor(out=t1[:], in0=res_t[:, dd, :],
                                        in1=rsb[:], op=OP.mult)
                t2 = lp.tile([128, GT], BF16, tag="t2")
                nc.vector.tensor_tensor(out=t2[:], in0=t1[:], in1=mrsb[:],
                                        op=OP.subtract)
                nc.vector.tensor_scalar(
                    out=h_g[:, dd, :], in0=t2[:],
                    scalar1=consts["ln1_s_c"][:, i, dd:dd + 1],
                    scalar2=consts["ln1_b_c"][:, i, dd:dd + 1],
                    op0=OP.mult, op1=OP.add)
                nc.scalar.copy(out=h8[g][:, dd, :], in_=h_g[:, dd, :])
            nc.sync.dma_start(out=h_dram[g][:], in_=h_g[:])

        # ------------------------------------------------------------------
        # FFN emitters (E = f1/fg+gate, F = f2+res, stats, applies)
        # ------------------------------------------------------------------
        def phase_e(sup, fp, hp2, wcf, fsq):
            f_s = fp.tile([128, FC, 1024], F8, tag="f")
            h_s = hp2.tile([128, DC, 1024], BF16, tag="hs")
            for sub in range(2):
                nc.gpsimd.dma_start(
                    out=h_s[:, :, sub * 512:(sub + 1) * 512],
                    in_=h_dram[sup * 2 + sub][:])
            for q4 in range(FC // 2):
                wf12_ch = wcf.tile([128, 2, DC, 256], F8, tag="wf12")
                nc.sync.dma_start(out=wf12_ch[:], in_=t["wf12"][i, q4])
                for cx in range(2):
                    fc = q4 * 2 + cx
                    cs = slice(cx * 128, (cx + 1) * 128)
                    for sub in range(2):
                        h8q = h8[sup * 2 + sub]
                        ps1 = mmp.tile([128, 512], F32, tag="mm")
                        psg = mmp.tile([128, 512], F32, tag="mm")
                        for j in range(DC // 2):
                            nc.tensor.matmul(
                                ps1[:], wf12_ch[:, 0, 2 * j:2 * j + 2, cs],
                                h8q[:, 2 * j:2 * j + 2, :],
                                start=(j == 0), stop=(j == DC // 2 - 1),
                                perf_mode=DR)
                        for j in range(DC // 2):
                            nc.tensor.matmul(
                                psg[:], wf12_ch[:, 1, 2 * j:2 * j + 2, cs],
                                h8q[:, 2 * j:2 * j + 2, :],
                                start=(j == 0), stop=(j == DC // 2 - 1),
                                perf_mode=DR)
                        sig = fsq.tile([128, 512], BF16, tag="fsig")
                        nc.scalar.activation(
                            out=sig[:], in_=psg[:], func=AF.Sigmoid,
                            bias=consts["bfg_c"][:, i, fc:fc + 1], scale=CFG)
                        nc.vector.scalar_tensor_tensor(
                            out=f_s[:, fc, sub * 512:(sub + 1) * 512],
                            in0=ps1[:],
                            scalar=consts["bf1_c"][:, i, fc:fc + 1],
                            in1=sig[:], op0=OP.add, op1=OP.mult)
            state[("f", sup)] = (f_s, h_s)

        def phase_f(sup, wc2, frp, fsq):
            f_s, h_s = state[("f", sup)]
            seg_res = {}
            for sub in range(2):
                seg_res[sub] = (
                    frp.tile([128, DC, 512], BF16, tag="res2", name="res2"),
                    fsq.tile([128, DC, 512], F8, tag="sq2", name="sq2"),
                )
            for dd in range(DC):
                wf2_ch = wc2.tile([128, FC, 128], F8, tag="wf2")
                nc.sync.dma_start(out=wf2_ch[:], in_=t["wf2"][i, dd])
                for sub in range(2):
                    res_t, sq_t = seg_res[sub]
                    ps2 = mmp.tile([128, 512], F32, tag="mm")
                    for j in range(FC // 2):
                        nc.tensor.matmul(
                            ps2[:], wf2_ch[:, 2 * j:2 * j + 2, :],
                            f_s[:, 2 * j:2 * j + 2,
                                sub * 512:(sub + 1) * 512],
                            start=(j == 0), stop=(j == FC // 2 - 1),
                            perf_mode=DR)
                    f2t = fsq.tile([128, 512], BF16, tag="f2t")
                    nc.scalar.activation(
                        out=f2t[:], in_=ps2[:], func=AF.Identity,
                        bias=consts["bf2_c"][:, i, dd:dd + 1], scale=CF2)
                    nc.vector.tensor_add(
                        out=res_t[:, dd, :], in0=f2t[:],
                        in1=h_s[:, dd, sub * 512:(sub + 1) * 512])
                    nc.gpsimd.tensor_tensor(out=sq_t[:, dd, :],
                                            in0=res_t[:, dd, :],
                                            in1=res_t[:, dd, :], op=OP.mult)
            state[("res2", sup)] = seg_res

        def stats_f(sup):
            seg_res = state[("res2", sup)]
            seg_st = {}
            for sub in range(2):
                res_t, sq_t = seg_res[sub]
                st = stp.tile([33, 512], F32, tag="st", name="st2")
                for dd in range(DC):
                    nc.tensor.matmul(st[0:1, :],
                                     sh["ones_col_bf"][:, 0:1],
                                     res_t[:, dd, :],
                                     start=(dd == 0), stop=(dd == DC - 1),
                                     skip_group_check=True)
                for dd in range(DC):
                    nc.tensor.matmul(st[32:33, :],
                                     sh["ones_col_8"][:, 0:1],
                                     sq_t[:, dd, :],
                                     start=(dd == 0), stop=(dd == DC - 1),
                                     skip_group_check=True)
                seg_st[sub] = st
            state[("stf", sup)] = seg_st

        def rows_f(sup):
            seg_st = state.pop(("stf", sup))
            rows = {}
            for sub in range(2):
                rs16, mrs16 = _ln_rows(nc, lrp, seg_st[sub], 1.0 / D,
                                       sh["eps1"], "a")
                rsb = lp.tile([128, 512], BF16, tag="rsb")
                nc.gpsimd.partition_broadcast(rsb[:], rs16)
                mrsb = lp.tile([128, 512], BF16, tag="mrsb")
                nc.gpsimd.partition_broadcast(mrsb[:], mrs16)
                rows[sub] = (rsb, mrsb)
            state[("rowf", sup)] = rows

        def applies_f(sup):
            seg_res = state.pop(("res2", sup))
            rows = state.pop(("rowf", sup))
            state.pop(("f", sup))
            for sub in range(2):
                seg = sup * 2 + sub
                res_t, sq_t = seg_res[sub]
                xg = xT[seg]
                rsb, mrsb = rows[sub]
                for dd in range(DC):
                    t1 = lp.tile([128, 512], BF16, tag="t1")
                    nc.vector.tensor_tensor(out=t1[:], in0=res_t[:, dd, :],
                                            in1=rsb[:], op=OP.mult)
                    t2 = lp.tile([128, 512], BF16, tag="t2")
                    nc.vector.tensor_tensor(out=t2[:], in0=t1[:], in1=mrsb[:],
                                            op=OP.subtract)
                    if SURVIVE[i] == 1.0:
                        nc.vector.tensor_scalar(
                            out=xg[:, dd, :], in0=t2[:],
                            scalar1=consts["ln2_s_c"][:, i, dd:dd + 1],
                            scalar2=consts["ln2_b_c"][:, i, dd:dd + 1],
                            op0=OP.mult, op1=OP.add)
                    else:
                        u = lp.tile([128, 512], BF16, tag="fu")
                        nc.vector.tensor_scalar(
                            out=u[:], in0=t2[:],
                            scalar1=consts["ln2_s_c"][:, i, dd:dd + 1],
                            scalar2=consts["ln2_b_c"][:, i, dd:dd + 1],
                            op0=OP.mult, op1=OP.add)
                        nc.vector.tensor_scalar_mul(
                            out=xg[:, dd, :], in0=xg[:, dd, :],
                            scalar1=1.0 - SURVIVE[i])
                        nc.vector.tensor_add(
                            out=xg[:, dd, :], in0=xg[:, dd, :], in1=u[:])
                if i < NL - 1:
                    nc.gpsimd.tensor_copy(out=xT8[seg][:], in_=xg[:])
                if i == NL - 1:
                    b0 = seg * GB
                    for dd in range(DC):
                        xv = xg[:, dd, :].rearrange("p (b l) -> p b l", l=L)
                        with nc.allow_low_precision(
                                reason="bf16 pooled sums within tolerance"):
                            nc.vector.tensor_reduce(
                                out=sh["pool_sum"][:, dd, b0:b0 + GB],
                                in_=xv,
                                axis=mybir.AxisListType.X, op=OP.add)
                        nc.vector.tensor_reduce(
                            out=sh["pool_max"][:, dd, b0:b0 + GB], in_=xv,
                            axis=mybir.AxisListType.X, op=OP.max)

        # ------------------------------------------------------------------
        # emission schedule
        # ------------------------------------------------------------------
        with (
            tc.tile_pool(name="wqk", bufs=1) as wp,
            tc.tile_pool(name="grp", bufs=2) as gp,
            tc.tile_pool(name="wch", bufs=3) as wc,
            tc.tile_pool(name="aog", bufs=2) as aop,
            tc.tile_pool(name="att", bufs=3) as at,
            tc.tile_pool(name="sq", bufs=2) as sqp,
        ):
            wq_s = wp.tile([128, DC, D], F8)
            nc.sync.dma_start(out=wq_s[:], in_=t["wq"][i])
            wk_s = wp.tile([128, DC, D], F8)
            nc.sync.dma_start(out=wk_s[:], in_=t["wk"][i])
            wv_s = wp.tile([128, DC, D], F8)
            nc.sync.dma_start(out=wv_s[:], in_=t["wv"][i])
            state["wqkv"] = (wq_s, wk_s, wv_s)

            for s in range(NG + 2):
                if 0 <= s - 3 < NG - 2:
                    stats(s - 3)
                    rowsD(s - 3)
                if s < NG:
                    stageA(s, wp, gp)
                if 0 <= s - 1 < NG:
                    stageB(s - 1, aop, at)
                if 0 <= s - 2 < NG:
                    stageC(s - 2, wc, sqp)
                if 0 <= s - 3 < NG - 2:
                    stageD(s - 3)
            stats(2)
            rowsD(2)
            stats(3)
            rowsD(3)
            state.pop("wqkv")

        with (
            tc.tile_pool(name="fbuf", bufs=1) as fp,
            tc.tile_pool(name="hsup", bufs=1) as hp2,
            tc.tile_pool(name="fwch", bufs=3) as wcf,
            tc.tile_pool(name="fsq", bufs=2) as fsq,
        ):
            phase_e(0, fp, hp2, wcf, fsq)
            stageD(2)
            stageD(3)
            hstp.release()
            rp.release()
            with (
                tc.tile_pool(name="fw2ch", bufs=3) as wc2,
                tc.tile_pool(name="fres", bufs=2) as frp,
            ):
                phase_f(0, wc2, frp, fsq)
                stats_f(0)
                rows_f(0)
                phase_e(1, fp, hp2, wcf, fsq)
                applies_f(0)
                phase_f(1, wc2, frp, fsq)
                stats_f(1)
                rows_f(1)
                applies_f(1)


def _head(nc, tc, t, consts, sh, mmp, srp, stp, y_out):
    pool_sum, pool_max = sh["pool_sum"], sh["pool_max"]
    ones_col_f, ones_row_f, eps1 = (sh["ones_col_f"], sh["ones_row_f"],
                                    sh["eps1"])
    with (
        tc.tile_pool(name="head", bufs=1) as hp,
    ):
        wd1_s = hp.tile([128, 17, 128], BF16)
        nc.sync.dma_start(out=wd1_s[:], in_=t["wd1"][:])
        # y1 = relu(pooled @ Wd1 + bd1), pooled = [mean(x), max(x), other];
        # 1/L for the mean is folded into the wd1 rows host-side, so the
        # matmul reads pool_sum / pool_max / otherT directly.
        ps1 = mmp.tile([128, 512], F32, tag="mm", name="hps")[:, 0:BPC]
        for c in range(17):
            if c < DC:
                rhs = pool_sum[:, c, :]
            elif c < 2 * DC:
                rhs = pool_max[:, c - DC, :]
            else:
                rhs = consts["otherT"][:]
            nc.tensor.matmul(ps1, wd1_s[:, c, :], rhs,
                             start=(c == 0), stop=(c == 16))
        y1 = hp.tile([128, BPC], F32R)
        nc.scalar.activation(out=y1[:], in_=ps1, func=AF.Relu,
                             bias=consts["bd1_c"][:], scale=1.0)

        # LN3 over the 128 features (partition dim)
        sq3 = hp.tile([128, BPC], F32R)
        nc.scalar.activation(out=sq3[:], in_=y1[:].bitcast(F32),
                             func=AF.Square)
        ps_s = stp.tile([33, 512], F32, tag="st", name="hst")
        nc.tensor.matmul(ps_s[0:1, 0:BPC], ones_col_f[:, 0:1],
                         y1[:], start=True, stop=True, skip_group_check=True)
        ps_q = srp.tile([65, 512], F32, tag="sum", name="hstq")
        nc.tensor.matmul(ps_q[0:1, 0:BPC], ones_col_f[:, 0:1],
                         sq3[:], start=True, stop=True, skip_group_check=True)
        with tc.tile_pool(name="hln", bufs=1) as lp:
            m = lp.tile([1, BPC], F32, name="hm")
            nc.vector.tensor_scalar_mul(out=m[:], in0=ps_s[0:1, 0:BPC],
                                        scalar1=1.0 / 128)
            e = lp.tile([1, BPC], F32, name="he")
            nc.vector.tensor_scalar(out=e[:], in0=ps_q[0:1, 0:BPC],
                                    scalar1=1.0 / 128, scalar2=EPS,
                                    op0=OP.mult, op1=OP.add)
            m2 = lp.tile([1, BPC], F32, name="hm2")
            nc.vector.tensor_tensor(out=m2[:], in0=m[:], in1=m[:],
                                    op=OP.mult)
            var = lp.tile([1, BPC], F32, name="hvar")
            nc.vector.tensor_tensor(out=var[:], in0=e[:], in1=m2[:],
                                    op=OP.subtract)
            sd = lp.tile([1, BPC], F32, name="hsd")
            nc.scalar.activation(out=sd[:], in_=var[:], func=AF.Sqrt,
                                 scale=1.0)
            rs16 = lp.tile([1, BPC], F32, name="hrs")
            nc.vector.reciprocal(out=rs16[:], in_=sd[:])
            mrs = lp.tile([1, BPC], F32, name="hmrs")
            nc.vector.tensor_tensor(out=mrs[:], in0=m[:], in1=rs16[:],
                                    op=OP.mult)
            rsb = lp.tile([128, BPC], F32, name="hrsb")
            nc.gpsimd.partition_broadcast(rsb[:], rs16[:])
            mrsb = lp.tile([128, BPC], F32, name="hmrsb")
            nc.gpsimd.partition_broadcast(mrsb[:], mrs[:])
            t1 = lp.tile([128, BPC], F32, name="ht1")
            nc.vector.tensor_tensor(out=t1[:], in0=y1[:].bitcast(F32),
                                    in1=rsb[:], op=OP.mult)
            t2 = lp.tile([128, BPC], F32, name="ht2")
            nc.vector.tensor_tensor(out=t2[:], in0=t1[:], in1=mrsb[:],
                                    op=OP.subtract)
            yln = hp.tile([128, BPC], BF16)
            nc.vector.tensor_scalar(
                out=yln[:], in0=t2[:], scalar1=consts["ln3_s_c"][:],
                scalar2=consts["ln3_b_c"][:], op0=OP.mult, op1=OP.add)

        # y2 = relu(yln @ Wd2 + bd2); y = y2 @ Wout + bout
        ps2 = mmp.tile([128, 512], F32, tag="mm", name="hps")[:, 0:BPC]
        nc.tensor.matmul(ps2, consts["wd2"][:], yln[:], start=True, stop=True)
        y2 = hp.tile([128, BPC], BF16)
        nc.scalar.activation(out=y2[:], in_=ps2, func=AF.Relu,
                             bias=consts["bd2_c"][:], scale=1.0)
        psy = mmp.tile([128, 512], F32, tag="mm", name="hps1")[0:1, 0:BPC]
        nc.tensor.matmul(psy, consts["wout"][:], y2[:], start=True, stop=True)
        yfin = hp.tile([1, BPC], F32)
        nc.vector.tensor_tensor(
            out=yfin[:], in0=psy,
            in1=consts["bout_t"][:].to_broadcast([1, BPC]), op=OP.add)
        nc.sync.dma_start(out=y_out[:], in_=yfin[:])


# ---------------------------------------------------------------------------
# host side
# ---------------------------------------------------------------------------

def _bf(x):
    return np.ascontiguousarray(np.asarray(x, np.float32)).astype(
        ml_dtypes.bfloat16)


def _f8(x, scale):
    a = np.asarray(x, np.float32) * np.float32(scale)
    np.clip(a, -224.0, 224.0, out=a)
    return np.ascontiguousarray(a).astype(ml_dtypes.float8_e4m3)


def _f32(x):
    return np.ascontiguousarray(np.asarray(x, np.float32))


def _prep_shared(I):
    """Weight transforms shared by all cores."""
    sv = np.array(SURVIVE, np.float32)
    s = {}
    s["w_in"] = _bf(I["W_in"])
    s["b_in_c"] = _f32(I["b_in"].reshape(DC, 128).T)
    for nm, W in (("wq", I["Wq"]), ("wk", I["Wk"]), ("wv", I["Wv"])):
        s[nm] = _f8(np.asarray(W, np.float32).reshape(NL, DC, 128, D)
                    .transpose(0, 2, 1, 3), SW)
    # wo+wg fused: [NL, DC(dd), 128, {o,g}, DC(kc), 128]
    og = [np.asarray(W, np.float32).reshape(NL, DC, 128, DC, 128)
          .transpose(0, 3, 2, 1, 4) for W in (I["Wo"], I["Wg"])]
    s["wog"] = np.ascontiguousarray(
        np.stack([_f8(og[0], SW), _f8(og[1], SW)], axis=3))
    # wf1+wfg fused: [NL, FC/2, 128, {f1,fg}, DC, 256]
    f1r = np.asarray(I["Wf1"], np.float32).reshape(NL, DC, 128, FC // 2, 256) \
        .transpose(0, 3, 2, 1, 4)
    fgr = np.asarray(I["Wfg"], np.float32).reshape(NL, DC, 128, FC // 2, 256) \
        .transpose(0, 3, 2, 1, 4)
    s["wf12"] = np.ascontiguousarray(
        np.stack([_f8(f1r, SW1), _f8(fgr, SW)], axis=3))
    # wf2: [NL, DC, 128, FC, 128]
    s["wf2"] = _f8(np.asarray(I["Wf2"], np.float32)
                   .reshape(NL, FC, 128, DC, 128)
                   .transpose(0, 3, 2, 1, 4), SW)

    def col(b, nch):
        return _f32(np.asarray(b).reshape(NL, nch, 128).transpose(2, 0, 1))

    s["bq_c"] = col(np.asarray(I["bq"], np.float32) * np.float32(SQ * QSCALE),
                    DC)
    s["bk_c"] = col(I["bk"], DC)
    s["bg_c"] = col(I["bg"], DC)
    # v-bias folds through the (linear) AV+o-proj: bo' = bv @ Wo + bo
    bo_f = np.asarray(I["bo"], np.float32) + np.einsum(
        "ld,ldc->lc", np.asarray(I["bv"], np.float32),
        np.asarray(I["Wo"], np.float32))
    s["bo_c"] = col(bo_f, DC)
    s["bf2_c"] = col(I["bf2"], DC)
    s["bf1_c"] = col(np.asarray(I["bf1"], np.float32) * np.float32(SW1), FC)
    s["bfg_c"] = col(I["bfg"], FC)
    s["ln1_s_c"] = col(I["ln1_s"], DC)
    s["ln1_b_c"] = col(I["ln1_b"], DC)
    s["ln2_s_c"] = col(I["ln2_s"] * sv[:, None], DC)
    s["ln2_b_c"] = col(I["ln2_b"] * sv[:, None], DC)
    s["ident4"] = _bf(np.tile(np.eye(128, dtype=np.float32), (1, 4)))
    wd1f = np.concatenate(
        [np.asarray(I["Wd1"], np.float32),
         np.zeros((17 * 128 - I["Wd1"].shape[0], 128), np.float32)],
        axis=0).reshape(17, 128, 128)
    wd1f[0:DC] *= np.float32(1.0 / L)   # mean pooling folds into Wd1
    s["wd1"] = _bf(wd1f.transpose(1, 0, 2))
    s["bd1_c"] = _f32(I["bd1"].reshape(128, 1))
    s["ln3_s_c"] = _f32(I["ln3_s"].reshape(128, 1))
    s["ln3_b_c"] = _f32(I["ln3_b"].reshape(128, 1))
    s["ln3_s_row"] = _f32(I["ln3_s"].reshape(1, 128))
    s["wd2"] = _bf(I["Wd2"])
    s["bd2_c"] = _f32(I["bd2"].reshape(128, 1))
    s["wout"] = _bf(I["Wout"])
    s["bout_t"] = _f32(I["bout"].reshape(1, 1))
    s["onesc"] = np.ones((128, 1), np.float32)
    s["onesr"] = np.ones((1, 128), np.float32)
    return s


def _prep_core(I, shared, c):
    m = dict(shared)
    cgm = np.asarray(I["cgm"], np.float32)
    m["cgmT"] = _bf(cgm[c * BPC:(c + 1) * BPC].reshape(NTOK, DFEAT).T)
    oth = np.asarray(I["other"], np.float32)[c * BPC:(c + 1) * BPC]  # [16,64]
    m["otherT"] = _bf(np.concatenate(
        [oth.T, np.zeros((128 - OTHER, BPC), np.float32)], axis=0))
    # rel-pos bias for the identity-matmul add:
    # battn[i, q, b_local, k] = SQ * rel[i, b_global - q + 127, k]
    rel = np.asarray(I["rel_emb"], np.float32)          # [NL, 255, 128]
    q_ar = np.arange(128)
    bat = np.empty((NL, 128, BPC, 128), np.float32)
    for bl in range(BPC):
        bg = c * BPC + bl
        idx = bg - q_ar + MAXPOS - 1                    # [128(q)]
        bat[:, :, bl, :] = rel[:, idx, :]               # [NL, 128, 128]
    m["battn"] = _bf(bat.transpose(1, 0, 2, 3) * np.float32(SQ))
    return m


def kernel(**inputs) -> np.ndarray:
    if "nc" not in _cached:
        _cached["nc"] = _build_nc()
    nc = _cached["nc"]
    shared = _prep_shared(inputs)
    in_maps = [_prep_core(inputs, shared, c) for c in range(NCORES)]
    res = run_bass_kernel_spmd(nc, in_maps, core_ids=list(range(NCORES)))
    y = np.concatenate([res.results[c]["y"].reshape(BPC)
                        for c in range(NCORES)])
    return y.reshape(B, 1).astype(np.float32)
